# revision 1
# baseline (speedup 1.0000x reference)
"""Trainium2 Bass kernel for the nn_Attention problem (v3).

v2 structure (host-pre-transposed x, W-stationary direct q^T/k^T,
2-tile supertiles) plus residual-compensated fp8 GEMMs:

    x = x8 + dx8 (both e4m3),  256*W = W8 + dW8 (both e4m3)
    x@W ~ (x8@W8 + dx8@W8 + x8@dW8) / 256     ("t3", ~bf16 accuracy)
        ~ (x8@W8 + dx8@W8) / 256              ("t2", W-quant error kept)

All fp8 matmuls run in DoubleRow perf mode (2 contraction rows/cycle,
256-deep per instruction).  Terms accumulate in a single PSUM group, so
a t3 GEMM costs 0.75x its bf16 version at the cost-model DoubleRow rate.

Modes: "bf16" | "t1" (naive fp8 qk, accuracy probe) | "t3" (q,k,v all
3-term) | "t2t3" (q 2-term, k/v 3-term).
"""

import sys

import numpy as np

for _p in ("/opt/trn_rl_repo",):
    if _p not in sys.path:
        sys.path.insert(0, _p)

import ml_dtypes  # noqa: E402

BF16 = ml_dtypes.bfloat16
F8E4 = ml_dtypes.float8_e4m3

HEAD_NUM = 16
HEAD_DIM = 128
INPUT_DIM = 1024
OUTPUT_DIM = 1024
QKV_DIM = 3 * HEAD_NUM * HEAD_DIM  # 6144
N_CORES = 8
B_TOTAL = 64 * 512
ROWS_PER_CORE = B_TOTAL // N_CORES  # 4096
P = 128
ST = 2                      # tiles per supertile
SP_ROWS = ST * P            # 256
WSCALE = 256.0

MODE = "qt1"

_PROG = None


def _mode_terms(mode):
    """(q_terms, k_terms, v_terms); 0 = bf16 GEMM."""
    return {
        "bf16": (0, 0, 0),
        "t1": (1, 1, 0),
        "qt1": (1, 0, 0),
        "t3": (3, 3, 3),
        "t2t3": (2, 3, 3),
        "t2": (2, 2, 3),
    }[mode]


def _build_program(n_tiles=ROWS_PER_CORE // P, repeats=1, mode=None,
                   ablate=()):
    """ablate: subset of {"out_dma", "v8_dma", "attention", "drains"} —
    produces a WRONG-output program for bottleneck isolation timing."""
    from contextlib import ExitStack

    import concourse.tile as tile
    from concourse import bacc, mybir

    if mode is None:
        mode = MODE
    ablate = set(ablate)
    qt_terms, kt_terms, v_terms = _mode_terms(mode)
    any_f8 = max(qt_terms, kt_terms, v_terms) > 0
    need_dx = max(qt_terms, kt_terms, v_terms) >= 2
    need_dw = max(qt_terms, kt_terms, v_terms) >= 3

    dt = mybir.dt
    Alu = mybir.AluOpType
    Act = mybir.ActivationFunctionType
    PM = mybir.MatmulPerfMode

    assert n_tiles % ST == 0
    n_super = n_tiles // ST
    rows = n_tiles * P
    nc = bacc.Bacc("TRN2", target_bir_lowering=False, debug=False,
                   num_devices=N_CORES)

    # --- dram inputs ---
    need_x16 = (qt_terms == 0 or kt_terms == 0 or v_terms == 0)
    w16_cols = (0 if qt_terms else 2048) + (0 if kt_terms else 2048) + \
        (0 if v_terms else 2048)
    if need_x16:
        xt_d = nc.dram_tensor("x_t", [INPUT_DIM, rows], dt.bfloat16,
                              kind="ExternalInput")
    if w16_cols:
        wpre_d = nc.dram_tensor("w_pre", [INPUT_DIM, w16_cols], dt.bfloat16,
                                kind="ExternalInput")
    w8_cols = 2048 * (1 + max((i for i, t in enumerate(
        (qt_terms, kt_terms, v_terms)) if t > 0), default=-1))
    if any_f8:
        x8_d = nc.dram_tensor("x_t8", [INPUT_DIM, rows], dt.float8e4,
                              kind="ExternalInput")
        w8_d = nc.dram_tensor("w_pre8", [INPUT_DIM, w8_cols], dt.float8e4,
                              kind="ExternalInput")
    if need_dx:
        dx8_d = nc.dram_tensor("dx_t8", [INPUT_DIM, rows], dt.float8e4,
                               kind="ExternalInput")
    if need_dw:
        dw8_d = nc.dram_tensor("dw_pre8", [INPUT_DIM, w8_cols], dt.float8e4,
                               kind="ExternalInput")
    bpreT_d = nc.dram_tensor("b_preT", [P, 32], dt.float32,
                             kind="ExternalInput")
    bprev_d = nc.dram_tensor("b_pre_v_rep", [P, 2048], dt.bfloat16,
                             kind="ExternalInput")
    wproj_d = nc.dram_tensor("w_proj", [HEAD_DIM, OUTPUT_DIM], dt.bfloat16,
                             kind="ExternalInput")
    bproj_d = nc.dram_tensor("b_proj16_rep", [P, OUTPUT_DIM], dt.float32,
                             kind="ExternalInput")
    mask01_d = nc.dram_tensor("mask01", [P, P], dt.bfloat16,
                              kind="ExternalInput")
    mask8_d = nc.dram_tensor("mask8", [P, 8], dt.bfloat16,
                             kind="ExternalInput")
    out_d = nc.dram_tensor("out", [rows, OUTPUT_DIM], dt.float32,
                           kind="ExternalOutput")

    KC = INPUT_DIM // P
    GROUPS = P // 8
    INV_SQRT_D = 1.0 / float(np.sqrt(HEAD_DIM))

    with tile.TileContext(nc) as tc, ExitStack() as ctx:
        consts = ctx.enter_context(tc.tile_pool(name="consts", bufs=1))
        xt_pool = ctx.enter_context(tc.tile_pool(name="xt", bufs=2))
        qt_pool = ctx.enter_context(tc.tile_pool(name="qt", bufs=2))
        kt_pool = ctx.enter_context(tc.tile_pool(name="kt", bufs=2))
        vsb_pool = ctx.enter_context(tc.tile_pool(name="vsb", bufs=2))
        v8_pool = ctx.enter_context(tc.tile_pool(name="v8", bufs=4))
        att_pool = ctx.enter_context(tc.tile_pool(name="att", bufs=6))
        ct_pool = ctx.enter_context(tc.tile_pool(name="ct", bufs=2))
        out_pool = ctx.enter_context(tc.tile_pool(name="outp", bufs=2))

        qk_ps = ctx.enter_context(
            tc.tile_pool(name="qk_ps", bufs=3, space="PSUM"))
        v_ps = ctx.enter_context(tc.tile_pool(name="v_ps", bufs=2,
                                              space="PSUM"))
        z_ps = ctx.enter_context(tc.tile_pool(name="z_ps", bufs=2,
                                              space="PSUM"))
        ct_ps = ctx.enter_context(tc.tile_pool(name="ct_ps", bufs=1,
                                               space="PSUM"))

        preloaded = {}

        def load_xt(s):
            r0 = s * SP_ROWS
            tiles = {}
            if need_x16:
                xt = xt_pool.tile([P, KC, SP_ROWS], dt.bfloat16, name="xt",
                                  tag="xt")
                for k in range(KC):
                    nc.sync.dma_start(
                        xt[:, k, :], xt_d[k * P:(k + 1) * P, r0:r0 + SP_ROWS])
                tiles["x16"] = xt
            if any_f8:
                x8 = xt_pool.tile([P, KC, SP_ROWS], dt.float8e4, name="x8",
                                  tag="x8")
                for k in range(KC):
                    # ACT queue: splits steady-state DMA load off the SP
                    # queue (x16 + half the v8 gathers + out stores)
                    nc.scalar.dma_start(
                        x8[:, k, :], x8_d[k * P:(k + 1) * P, r0:r0 + SP_ROWS])
                tiles["x8"] = x8
            if need_dx:
                dx8 = xt_pool.tile([P, KC, SP_ROWS], dt.float8e4, name="dx8",
                                   tag="dx8")
                for k in range(KC):
                    nc.sync.dma_start(
                        dx8[:, k, :],
                        dx8_d[k * P:(k + 1) * P, r0:r0 + SP_ROWS])
                tiles["dx8"] = dx8
            return tiles

        # ---- resident constants ----
        def load_w(dram, cols, cdt, tag):
            # unique tag per W tensor: untagged tiles from one call site
            # share a slot, which deadlocks a bufs=1 pool
            sb = consts.tile([P, KC, cols], cdt, name=tag, tag=tag)
            for k in range(KC):
                eng = nc.sync if k % 2 == 0 else nc.scalar
                eng.dma_start(sb[:, k, :], dram[k * P:(k + 1) * P, :])
            return sb

        wpre_sb = load_w(wpre_d, w16_cols, dt.bfloat16, "w16_sb") \
            if w16_cols else None
        w8_sb = load_w(w8_d, w8_cols, dt.float8e4, "w8_sb") if any_f8 else None
        dw8_sb = load_w(dw8_d, w8_cols, dt.float8e4, "dw8_sb") \
            if need_dw else None
        wproj_sb = consts.tile([P, OUTPUT_DIM], dt.bfloat16)
        nc.sync.dma_start(wproj_sb[:], wproj_d[:, :])
        bpreT_sb = consts.tile([P, 32], dt.float32)
        nc.sync.dma_start(bpreT_sb[:], bpreT_d[:, :])
        bprev_sb = consts.tile([P, 2048], dt.bfloat16)
        nc.sync.dma_start(bprev_sb[:], bprev_d[:, :])
        bproj_sb = consts.tile([P, OUTPUT_DIM], dt.float32)
        nc.sync.dma_start(bproj_sb[:], bproj_d[:, :])
        mask01_sb = consts.tile([P, P], dt.bfloat16)
        nc.sync.dma_start(mask01_sb[:], mask01_d[:, :])
        mask8_sb = consts.tile([P, 8], dt.bfloat16)
        nc.sync.dma_start(mask8_sb[:], mask8_d[:, :])

        # supertile 0's x loads early (right after consts) so the first
        # matmuls don't additionally wait on supertile-order emission.
        preloaded[0] = load_xt(0)

        # bf16-W column offsets within the packed w16 tensor
        w16_off = {}
        off = 0
        for part, terms in (("q", qt_terms), ("k", kt_terms),
                            ("v", v_terms)):
            if terms == 0:
                w16_off[part] = off
                off += 2048

        state = {}

        def front_gen(s):
            xts = preloaded.pop(s) if s in preloaded else load_xt(s)

            qt = qt_pool.tile([P, SP_ROWS, HEAD_NUM], dt.bfloat16, name="qt")
            kt = kt_pool.tile([P, SP_ROWS, HEAD_NUM], dt.bfloat16, name="kt")
            for jc in range(32):
                part = "q" if jc < 16 else "k"
                terms = qt_terms if jc < 16 else kt_terms
                pst = qk_ps.tile([P, SP_ROWS], dt.float32, name="qkps",
                                 tag="qkps")
                ps = pst[:]
                if terms == 0:
                    j0 = w16_off[part] + (jc % 16) * P
                    for k in range(KC):
                        nc.tensor.matmul(
                            ps, lhsT=wpre_sb[:, k, j0:j0 + P],
                            rhs=xts["x16"][:, k, :],
                            start=(k == 0), stop=(k == KC - 1))
                        yield
                else:
                    j0 = jc * P
                    ops = [(w8_sb, "x8"), (w8_sb, "dx8"), (dw8_sb, "x8")]
                    for ti in range(terms):
                        wsb, xk = ops[ti]
                        for kk in range(KC // 2):
                            nc.tensor.matmul(
                                ps,
                                lhsT=wsb[:, 2 * kk:2 * kk + 2, j0:j0 + P],
                                rhs=xts[xk][:, 2 * kk:2 * kk + 2, :],
                                start=(ti == 0 and kk == 0),
                                stop=(ti == terms - 1 and kk == KC // 2 - 1),
                                perf_mode=PM.DoubleRow)
                            yield
                if "drains" in ablate:
                    continue
                dst = qt if jc < 16 else kt
                h = jc % 16
                scale = (1.0 / WSCALE) if terms else 1.0
                if jc % 2 == 0:
                    nc.scalar.activation(dst[:, :, h], ps, Act.Identity,
                                         bias=bpreT_sb[:, jc:jc + 1],
                                         scale=scale)
                else:
                    nc.vector.tensor_scalar(dst[:, :, h], ps, scale,
                                            bpreT_sb[:, jc:jc + 1],
                                            Alu.mult, Alu.add)

            v8s = []
            for t2 in range(ST):
                vsb = vsb_pool.tile([P, 2048], dt.bfloat16, name="vsb",
                                    tag="vsb")
                if v_terms == 0:
                    for c in range(4):
                        ps = v_ps.tile([P, 512], dt.float32, name="vps",
                                       tag="vps")
                        j0 = w16_off["v"] + c * 512
                        for k in range(KC):
                            nc.tensor.matmul(
                                ps[:],
                                lhsT=xts["x16"][:, k, t2 * P:(t2 + 1) * P],
                                rhs=wpre_sb[:, k, j0:j0 + 512],
                                start=(k == 0), stop=(k == KC - 1))
                            yield
                        if "drains" not in ablate:
                            nc.vector.tensor_tensor(
                                vsb[:, c * 512:(c + 1) * 512], ps[:],
                                bprev_sb[:, c * 512:(c + 1) * 512], Alu.add)
                else:
                    ops = [("x8", w8_sb), ("dx8", w8_sb), ("x8", dw8_sb)]
                    for c in range(8):
                        ps = v_ps.tile([P, 256], dt.float32, name="vps",
                                       tag="vps")
                        j0 = 4096 + c * 256
                        for ti in range(v_terms):
                            xk, wsb = ops[ti]
                            for kk in range(KC // 2):
                                nc.tensor.matmul(
                                    ps[:],
                                    lhsT=xts[xk][:, 2 * kk:2 * kk + 2,
                                                 t2 * P:(t2 + 1) * P],
                                    rhs=wsb[:, 2 * kk:2 * kk + 2,
                                            j0:j0 + 256],
                                    start=(ti == 0 and kk == 0),
                                    stop=(ti == v_terms - 1
                                          and kk == KC // 2 - 1),
                                    perf_mode=PM.DoubleRow)
                                yield
                        if "drains" not in ablate:
                            nc.vector.scalar_tensor_tensor(
                                out=vsb[:, c * 256:(c + 1) * 256], in0=ps[:],
                                scalar=1.0 / WSCALE,
                                in1=bprev_sb[:, c * 256:(c + 1) * 256],
                                op0=Alu.mult, op1=Alu.add)
                v8 = v8_pool.tile([P, GROUPS, HEAD_DIM], dt.bfloat16,
                                  name="v8", tag="v8")
                if "v8_dma" not in ablate:
                    for g in range(GROUPS):
                        eng = nc.gpsimd if g % 2 else nc.sync
                        eng.dma_start(
                            v8[:, g, :],
                            vsb[8 * g:8 * g + 8, :].rearrange(
                                "b (g d) -> b g d", d=HEAD_DIM),
                        )
                else:
                    nc.gpsimd.memset(v8[:], 0.0)
                v8s.append(v8)
            state[s] = (qt, kt, v8s)

        def attention_tile(qt, kt, v8, b_off, r0):
            ct_box = {}
            zs, ems, rbfs, sds = {}, {}, {}, {}
            LAG2 = 4   # mm1 -> mm2 (z-pair p is last read at step 2p+1+LAG2
            LAG3 = 5   # and pair p+3 reallocs its bank at step 2p+6, so
            #            LAG2 must stay < 5 or the queues deadlock)

            def mm1(g):
                if g == 0:
                    ct_box["ct"] = ct_ps.tile([P, 2, P], dt.float32,
                                              name="ct", tag="ct")
                z8 = z_ps.tile([P, P], dt.float32, name="z8", tag="z8")
                zs[g] = z8[:]
                b0 = b_off + 8 * g
                nc.tensor.matmul(
                    zs[g],
                    lhsT=qt[:, b0:b0 + 8, :].rearrange("d b h -> d (b h)"),
                    rhs=kt[:, b0:b0 + 8, :].rearrange("d b h -> d (b h)"),
                    start=True,
                    stop=True,
                    skip_group_check=True,
                )
                em_raw = att_pool.tile([P, P], dt.bfloat16, tag="emr",
                                       name="em_raw")
                nc.scalar.activation(em_raw[:], zs[g], Act.Exp,
                                     scale=INV_SQRT_D)
                em = att_pool.tile([P, P], dt.bfloat16, tag="em", name="em")
                den = att_pool.tile([P, 1], dt.float32, tag="den", name="den")
                nc.vector.scalar_tensor_tensor(
                    out=em[:], in0=em_raw[:], scalar=1.0, in1=mask01_sb[:],
                    op0=Alu.mult, op1=Alu.mult, accum_out=den[:])
                ems[g] = em
                rbf = att_pool.tile([P, 1], dt.bfloat16, tag="rbf",
                                    name="rbf")
                with nc.allow_low_precision(reason="softmax recip to bf16"):
                    nc.vector.reciprocal(rbf[:], den[:])
                rbfs[g] = rbf

            def mm2(g):
                # sigma scratch lives in the spare lane of the ct bank so
                # the z8 slot frees right after exp() reads it
                sig = ct_box["ct"][:, 1, g:g + 1]
                nc.tensor.matmul(sig, lhsT=ems.pop(g)[:], rhs=rbfs.pop(g)[:],
                                 start=True, stop=True, skip_group_check=True)
                sd = att_pool.tile([P, 8], dt.bfloat16, tag="sd", name="sd")
                nc.vector.tensor_scalar(sd[:], mask8_sb[:], sig, None,
                                        Alu.mult)
                sds[g] = sd
                zs.pop(g)

            def mm3(g):
                b0 = 8 * g
                nc.tensor.matmul(ct_box["ct"][:, 0, b0:b0 + 8],
                                 lhsT=v8[:, g, :],
                                 rhs=sds.pop(g)[:], start=True, stop=True,
                                 skip_group_check=True)

            steps = []
            for i in range(GROUPS + LAG3):
                def step(i=i):
                    if i < GROUPS:
                        mm1(i)
                    if LAG2 <= i < GROUPS + LAG2:
                        mm2(i - LAG2)
                    if LAG3 <= i < GROUPS + LAG3:
                        mm3(i - LAG3)
                steps.append(step)

            def tail():
                ct_sb = ct_pool.tile([P, P], dt.bfloat16, name="ct_sb")
                nc.scalar.copy(ct_sb[:], ct_box["ct"][:, 0, :])
                out_sb = out_pool.tile([P, OUTPUT_DIM], dt.float32,
                                       name="out_sb")
                pw = 512 if v_terms == 0 else 256
                for c in range(OUTPUT_DIM // pw):
                    o_ps = v_ps.tile([P, pw], dt.float32, name="o_ps",
                                     tag="vps")
                    nc.tensor.matmul(
                        o_ps[:],
                        lhsT=ct_sb[:],
                        rhs=wproj_sb[:, c * pw:(c + 1) * pw],
                        start=True,
                        stop=True,
                    )
                    nc.vector.tensor_tensor(
                        out_sb[:, c * pw:(c + 1) * pw],
                        o_ps[:],
                        bproj_sb[:, c * pw:(c + 1) * pw],
                        Alu.add,
                    )
                if "out_dma" not in ablate or r0 == 0:
                    eng = nc.sync if (r0 // P) % 2 == 0 else nc.scalar
                    eng.dma_start(out_d[r0:r0 + P, :], out_sb[:])

            steps.append(tail)
            return steps

        def attention_super(s):
            qt, kt, v8s = state.pop(s)
            steps = []
            for t2 in range(ST):
                steps.extend(attention_tile(qt, kt, v8s[t2], t2 * P,
                                            s * SP_ROWS + t2 * P))
            return steps

        n_front = 16 * (qt_terms * 4 or KC) + 16 * (kt_terms * 4 or KC) \
            + ST * (8 * v_terms * 4 or 4 * KC)
        n_steps = ST * (GROUPS + 6)
        ivl = max(1, n_front // n_steps)

        def emit_pass():
            if "attention" in ablate:
                dummy = out_pool.tile([P, OUTPUT_DIM], dt.float32,
                                      name="dummy_out")
                nc.vector.memset(dummy[:], 0.0)
                for s in range(n_super):
                    for _ in front_gen(s):
                        pass
                    state.pop(s)
                nc.sync.dma_start(out_d[0:P, :], dummy[:])
                return
            prev = None
            for s in range(n_super):
                steps = attention_super(prev) if prev is not None else []
                si = 0
                yi = 0
                for _ in front_gen(s):
                    yi += 1
                    if si < len(steps) and yi % ivl == 0:
                        steps[si]()
                        si += 1
                while si < len(steps):
                    steps[si]()
                    si += 1
                prev = s
            for step in attention_super(prev):
                step()

        for _r in range(repeats):
            emit_pass()

    nc.compile()
    return nc


def _host_inputs(x, W_pre, b_pre, W_proj, b_proj, n_tiles=ROWS_PER_CORE // P,
                 n_cores=N_CORES, mode=None):
    if mode is None:
        mode = MODE
    qt_terms, kt_terms, v_terms = _mode_terms(mode)
    any_f8 = max(qt_terms, kt_terms, v_terms) > 0
    need_dx = max(qt_terms, kt_terms, v_terms) >= 2
    need_dw = max(qt_terms, kt_terms, v_terms) >= 3
    need_x16 = (qt_terms == 0 or kt_terms == 0 or v_terms == 0)

    rows = n_tiles * P
    xf = np.asarray(x, dtype=np.float32).reshape(-1, INPUT_DIM)
    xT = np.ascontiguousarray(xf.T)
    wpre32 = np.asarray(W_pre, dtype=np.float32)
    wproj16 = np.asarray(W_proj, dtype=np.float32).astype(BF16)
    bpre32 = np.asarray(b_pre, dtype=np.float32)
    bpreT = np.ascontiguousarray(
        bpre32[:4096].reshape(32, P).T.astype(np.float32))
    bprev_rep = np.broadcast_to(bpre32[4096:].astype(BF16)[None, :],
                                (P, 2048)).copy()
    bproj_rep = np.broadcast_to(
        (16.0 * np.asarray(b_proj, dtype=np.float32))[None, :],
        (P, OUTPUT_DIM)).copy()
    pi = np.arange(P)[:, None] // HEAD_NUM
    fi = np.arange(P)[None, :] // HEAD_NUM
    mask01 = (pi == fi).astype(BF16)
    mask8 = (np.arange(P)[:, None] // HEAD_NUM
             == np.arange(8)[None, :]).astype(BF16)

    base = {
        "b_preT": bpreT,
        "b_pre_v_rep": bprev_rep,
        "w_proj": wproj16,
        "b_proj16_rep": bproj_rep,
        "mask01": mask01,
        "mask8": mask8,
    }
    if need_x16:
        xT16_all = xT.astype(BF16)
    w16_parts = []
    for part, terms in (("q", qt_terms), ("k", kt_terms), ("v", v_terms)):
        if terms == 0:
            i = {"q": 0, "k": 1, "v": 2}[part]
            w16_parts.append(wpre32[:, i * 2048:(i + 1) * 2048].astype(BF16))
    if w16_parts:
        base["w_pre"] = np.ascontiguousarray(np.concatenate(w16_parts,
                                                            axis=1))
    if any_f8:
        w8_cols = 2048 * (1 + max(i for i, t in enumerate(
            (qt_terms, kt_terms, v_terms)) if t > 0))
        ws = (wpre32[:, :w8_cols] * WSCALE).astype(F8E4)
        base["w_pre8"] = ws
        x8_all = xT.astype(F8E4)
    if need_dw:
        base["dw_pre8"] = (wpre32[:, :w8_cols] * WSCALE
                           - ws.astype(np.float32)).astype(F8E4)
    if need_dx:
        dx8_all = (xT - x8_all.astype(np.float32)).astype(F8E4)

    in_maps = []
    for c in range(n_cores):
        m = dict(base)
        sl = np.s_[:, c * rows:(c + 1) * rows]
        if need_x16:
            m["x_t"] = np.ascontiguousarray(xT16_all[sl])
        if any_f8:
            m["x_t8"] = np.ascontiguousarray(x8_all[sl])
        if need_dx:
            m["dx_t8"] = np.ascontiguousarray(dx8_all[sl])
        in_maps.append(m)
    return in_maps


def kernel(x, W_pre, b_pre, W_proj, b_proj):
    global _PROG
    from concourse.bass_utils import run_bass_kernel_spmd

    if _PROG is None:
        _PROG = _build_program()

    in_maps = _host_inputs(x, W_pre, b_pre, W_proj, b_proj)
    res = run_bass_kernel_spmd(_PROG, in_maps, list(range(N_CORES)))
    out = np.concatenate([res.results[c]["out"] for c in range(N_CORES)],
                         axis=0)
    return out.reshape(*np.asarray(x).shape[:-1], OUTPUT_DIM).astype(np.float32)


def _make_sharded_fn(nc, n_cores=N_CORES):
    """Replicates bass2jax.run_bass_via_pjrt's multi-core path but without
    donation, so inputs can be staged on device once and execution timed
    across repeated calls (dev/benchmark helper, unused by grading)."""
    import jax
    from jax.sharding import Mesh, PartitionSpec, NamedSharding
    from jax.experimental.shard_map import shard_map
    from concourse import mybir
    from concourse.bass2jax import (_bass_exec_p, install_neuronx_cc_hook,
                                    partition_id_tensor)

    install_neuronx_cc_hook()
    in_names, out_names, out_avals = [], [], []
    for alloc in nc.m.functions[0].allocations:
        if not isinstance(alloc, mybir.MemoryLocationSet):
            continue
        name = alloc.memorylocations[0].name
        if alloc.kind == "ExternalInput":
            in_names.append(name)
        elif alloc.kind == "ExternalOutput":
            out_names.append(name)
            out_avals.append(jax.core.ShapedArray(
                tuple(alloc.tensor_shape), mybir.dt.np(alloc.dtype)))
    partition_name = (nc.partition_id_tensor.name
                      if nc.partition_id_tensor else None)
    if partition_name in in_names:
        in_names.remove(partition_name)
    n_params = len(in_names)
    all_names = list(in_names) + list(out_names)
    if partition_name is not None:
        all_names.append(partition_name)

    def _body(*args):
        operands = list(args)
        if partition_name is not None:
            operands.append(partition_id_tensor())
        return tuple(_bass_exec_p.bind(
            *operands,
            out_avals=tuple(out_avals),
            in_names=tuple(all_names),
            out_names=tuple(out_names),
            lowering_input_output_aliases=(),
            sim_require_finite=True,
            sim_require_nnan=True,
            nc=nc,
        ))

    devices = jax.devices()[:n_cores]
    mesh = Mesh(np.asarray(devices), ("core",))
    spec = PartitionSpec("core")
    fn = jax.jit(shard_map(_body, mesh=mesh,
                           in_specs=(spec,) * (n_params + len(out_names)),
                           out_specs=(spec,) * len(out_names),
                           check_rep=False))
    sharding = NamedSharding(mesh, spec)
    return fn, in_names, out_names, out_avals, sharding


def run_timed(nc, in_maps, iters=10):
    import time as _time
    import jax

    n_cores = len(in_maps)
    fn, in_names, out_names, out_avals, sharding = _make_sharded_fn(nc, n_cores)
    dev_in = [
        jax.device_put(
            np.concatenate([np.asarray(in_maps[c][nm])
                            for c in range(n_cores)], axis=0), sharding)
        for nm in in_names
    ]
    dev_zero = [
        jax.device_put(
            np.zeros((n_cores * av.shape[0], *av.shape[1:]), av.dtype),
            sharding)
        for av in out_avals
    ]
    outs = fn(*dev_in, *dev_zero)
    jax.block_until_ready(outs)
    times = []
    for _ in range(iters):
        t0 = _time.perf_counter()
        outs = fn(*dev_in, *dev_zero)
        jax.block_until_ready(outs)
        times.append(_time.perf_counter() - t0)
    results = [
        {nm: np.asarray(outs[i]).reshape(n_cores, *out_avals[i].shape)[c]
         for i, nm in enumerate(out_names)}
        for c in range(n_cores)
    ]
    return results, times

